# revision 1
# baseline (speedup 1.0000x reference)
"""GNN message passing (scatter-add of gathered edge features) on 8 TRN2 cores.

out[n] = sum over edges (s,d) with d==n of x[s].

Design: dst nodes are split across 8 cores (12500 each). On the host,
each core's nodes are sorted by in-degree and packed into 128-node
chunks; every node in a chunk gets exactly S slots (S = cross-core max
degree of that chunk, rounded up to even), and the gathered x[src]
rows (fp16) are packed slot-major: [128 partitions | chunk-group |
S slots | 32 feats], zero-padded. Chunks of equal-ish S form groups;
consecutive groups share big (~2 MB) DMA loads. Per group the device
does up to 4 DVE tree-add halvings over the slot axis (fp16 2x mode,
contiguous halves) plus a short add chain for any odd remainder,
writing into a per-batch stage tile that is stored with one DMA per
batch (input loads on the Sync HWDGE ring, output stores on the
Scalar ring). No index math, no one-hots, no matmuls on the device.
Measured (neuron-profile, 20-rep body): 41.6 us/exec steady state,
DMA 96% busy at ~356 GB/s (per-core HBM roofline), DVE 92%.
"""
import sys
import numpy as np

sys.path.insert(0, '/opt/trn_rl_repo')

N = 100000
D = 32
NC = 8
NPC = N // NC                  # 12500 dst nodes per core
CH = 128                       # nodes per chunk (one per partition)
NCHUNK = -(-NPC // CH)         # 98 chunks per core
NNP = NCHUNK * CH              # 12544 padded nodes per core
FMAX = 8192                    # max per-partition elems (fp16) per group
GMAX = 16                      # max chunks per group
BMAX = 8192                    # max per-partition elems per DMA batch

_cache = {}


def _build(groups, reps=1, loop_n=0):
    import concourse.bacc as bacc
    import concourse.tile as tile
    import concourse.mybir as mybir

    nc = bacc.Bacc("TRN2", target_bir_lowering=False, debug=False,
                   num_devices=NC)
    f16 = mybir.dt.float16
    F = sum(G * S * D for G, S in groups)

    xj = nc.dram_tensor("xj", (128, F), f16, kind="ExternalInput").ap()
    y = nc.dram_tensor("y", (128, NCHUNK * D), f16,
                       kind="ExternalOutput").ap()

    # plan DMA batches: consecutive groups, <= BMAX elems per partition
    batches = []  # (dram_off, width, [(tile_off, G, S), ...])
    cur = []
    cur_w = 0
    off = 0
    for (G, S) in groups:
        W = G * S * D
        if cur and cur_w + W > BMAX:
            batches.append((off - cur_w, cur_w, cur))
            cur, cur_w = [], 0
        cur.append((cur_w, G, S))
        cur_w += W
        off += W
    if cur:
        batches.append((off - cur_w, cur_w, cur))

    add = mybir.AluOpType.add
    with tile.TileContext(nc) as tc:
        with (
            tc.tile_pool(name="xt", bufs=3) as xpool,
            tc.tile_pool(name="t1", bufs=2) as t1pool,
            tc.tile_pool(name="t2", bufs=2) as t2pool,
            tc.tile_pool(name="t3", bufs=2) as t3pool,
            tc.tile_pool(name="t4", bufs=2) as t4pool,
            tc.tile_pool(name="ac", bufs=2) as apool,
            tc.tile_pool(name="st", bufs=3) as spool,
        ):
            def body():
                for _ in range(reps):
                    c0 = 0
                    for (doff, bw, glist) in batches:
                        xt = xpool.tile([128, bw], f16, tag="xt")
                        nc.sync.dma_start(xt[:], xj[:, doff:doff + bw])
                        bG = sum(G for _, G, _ in glist)
                        st = spool.tile([128, bG, D], f16, tag="st")
                        g0 = 0
                        for (toff, G, S) in glist:
                            src = xt[:, toff:toff + G * S * D].rearrange(
                                "p (g w) -> p g w", g=G)
                            w = S
                            pools = [t1pool, t2pool, t3pool, t4pool]
                            lvl = 0
                            pend = []   # odd leftover slots, added at end
                            while w >= 2 and lvl < 4:
                                if w % 2 == 1:
                                    pend.append(
                                        src[:, :, (w - 1) * D:w * D])
                                    w -= 1
                                h = w // 2
                                t = pools[lvl].tile(
                                    [128, G, h * D], f16,
                                    tag=f"t{lvl + 1}")
                                nc.vector.tensor_tensor(
                                    t[:], src[:, :, 0:h * D],
                                    src[:, :, h * D:w * D], add)
                                src = t[:]
                                w = h
                                lvl += 1
                            # remaining terms: w leading slots + pend
                            terms = [src[:, :, s * D:(s + 1) * D]
                                     for s in range(w)] + pend
                            stg = st[:, g0:g0 + G, :]
                            if len(terms) == 1:
                                nc.vector.tensor_copy(stg, terms[0])
                            else:
                                acc = terms[0]
                                for i, term in enumerate(terms[1:]):
                                    if i < len(terms) - 2:
                                        nxt = apool.tile(
                                            [128, G, D], f16, tag="ac")
                                        nc.vector.tensor_tensor(
                                            nxt[:, :, 0:D], acc, term,
                                            add)
                                        acc = nxt[:, :, 0:D]
                                    else:
                                        nc.vector.tensor_tensor(
                                            stg, acc, term, add)
                            g0 += G
                        nc.scalar.dma_start(
                            y[:, c0 * D:(c0 + bG) * D], st[:])
                        c0 += bG

            if loop_n:
                with tc.For_i(0, loop_n, 1,
                              hint_engines=(mybir.EngineType.DVE,)):
                    body()
            else:
                body()

    nc.compile()
    return nc


def _structure(deg_sorted):
    """deg_sorted: [NC, NNP] per-core degrees in descending order.
    Returns the common groups structure."""
    chunk_max = deg_sorted[:, ::CH].max(axis=0)        # [NCHUNK]
    S_pad = np.maximum(1, chunk_max).astype(np.int64)
    groups = []
    i = 0
    while i < NCHUNK:
        S = int(S_pad[i])
        j = i + 1
        while (j < NCHUNK and (j - i + 1) * S * D <= FMAX
               and (j - i + 1) <= GMAX
               and S - int(S_pad[j]) <= max(1, S // 16)):
            j += 1
        groups.append((j - i, S))
        i = j
    return tuple(groups)


def _prep_inputs(x, edge_index):
    """Returns (in_maps, groups, perms)."""
    x = np.ascontiguousarray(np.asarray(x), dtype=np.float32)
    ei = np.asarray(edge_index)
    src = ei[0].astype(np.int64)
    dst = ei[1].astype(np.int64)
    xh = np.zeros((N + 1, D), np.float16)
    xh[:N] = x.astype(np.float16)

    core = dst // NPC
    per_core = []
    perms = []
    deg_sorted = np.zeros((NC, NNP), np.int64)
    for k in range(NC):
        m = core == k
        s_k = src[m]
        d_k = dst[m] - k * NPC
        deg = np.zeros(NNP, np.int64)
        deg[:NPC] = np.bincount(d_k, minlength=NPC)
        perm = np.argsort(-deg, kind="stable")   # node ids, degree desc
        deg_sorted[k] = deg[perm]
        perms.append(perm)
        per_core.append((s_k, d_k))

    groups = _structure(deg_sorted)

    # per-sorted-position chunk column base and S (slot-major layout)
    colbase = np.zeros(NNP, np.int64)
    off = 0
    c0 = 0
    for (G, S) in groups:
        for ci in range(G):
            c = c0 + ci
            colbase[c * CH:(c + 1) * CH] = off + ci * S * D
        off += G * S * D
        c0 += G
    F = off

    feat_idx = np.arange(D, dtype=np.int64)[None, :]
    in_maps = []
    for k in range(NC):
        s_k, d_k = per_core[k]
        perm = perms[k]
        pos = np.empty(NNP, np.int64)
        pos[perm] = np.arange(NNP)
        q = pos[d_k]                       # sorted position per edge
        order = np.argsort(q, kind="stable")
        qo = q[order]
        so = s_k[order]
        cnts = np.bincount(qo, minlength=NNP)
        cum = np.concatenate(([0], np.cumsum(cnts)))
        slot = np.arange(len(qo), dtype=np.int64) - cum[qo]
        p = qo % CH
        cols = (colbase[qo] + slot * D)[:, None] + feat_idx
        xjk = np.zeros((128, F), np.float16)
        xjk[p[:, None], cols] = xh[so]
        in_maps.append({"xj": xjk})
    return in_maps, groups, perms


def kernel(x, edge_index):
    from concourse import bass_utils

    in_maps, groups, perms = _prep_inputs(x, edge_index)
    if groups not in _cache:
        _cache[groups] = _build(groups)
    nc = _cache[groups]

    res = None
    for attempt in range(3):
        try:
            res = bass_utils.run_bass_kernel_spmd(nc, in_maps,
                                                  core_ids=list(range(NC)))
            break
        except Exception:
            if attempt == 2:
                raise
    out = np.empty((N, D), np.float32)
    for k in range(NC):
        yk = np.asarray(res.results[k]["y"]).reshape(128, NCHUNK, D)
        yk = yk.transpose(1, 0, 2).reshape(NNP, D)
        perm = perms[k]
        valid = perm < NPC
        out[k * NPC + perm[valid]] = yk[valid]
    return out



# revision 3
# speedup vs baseline: 1.5182x; 1.5182x over previous
"""GNN message passing (scatter-add of gathered edge features) on 8 TRN2 cores.

out[n] = sum over edges (s,d) with d==n of x[s].

Design (v2, fp8e3 + PE identity-matmul segment sum): dst nodes are split
across 8 cores (12500 each) and sorted by in-degree; groups of 128
consecutive sorted nodes map to the 128 PSUM partitions. Edge features
x[src] are quantized host-side to float8_e3m4 (4 mantissa bits; rel RMS
~1.3e-2) and packed plane-major: plane p holds, for every group g with
S_g > p (a prefix, since groups are degree-sorted), the 128x32 block of
slot-p edge rows. The device streams the planes through the PE as
moving data against a resident 128x128 fp8 identity (lhsT), accumulating
into PSUM fp32: psum[u, g*32+d] += plane_p[u, g*32+d]. All 98*32=3136
output columns live in 7 PSUM banks; when the last plane touching a
bank retires, the bank is copied (fp32->fp16) to an SBUF stage tile
(copies alternate DVE/Act) and the stage is stored with one DMA per
rep. fp8 halves HBM traffic vs fp16 (6.5 MB/core, 2% padding) and the
reduction rides the otherwise-idle PE at 1 col/cycle; fp32 PSUM
accumulation of e3m4 values is exact.
"""
import sys
import numpy as np
import ml_dtypes

sys.path.insert(0, '/opt/trn_rl_repo')

N = 100000
D = 32
NC = 8
NPC = N // NC                  # 12500 dst nodes per core
CH = 128                       # nodes per group (one per partition)
NCHUNK = -(-NPC // CH)         # 98 groups per core
NNP = NCHUNK * CH              # 12544 padded nodes per core
YC = NCHUNK * D                # 3136 output cols
BANK = 512                     # psum bank cols (fp32)
BMAX = 8192                    # stream bytes per partition per DMA batch
F8 = ml_dtypes.float8_e3m4
FP8_MAX = 15.49

_cache = {}


def _plan(S_g):
    """pieces/batches for the plane-major fp8 stream."""
    Smax = max(S_g)
    n_p = [sum(1 for s in S_g if s > p) for p in range(Smax)]
    off = []
    F = 0
    for p in range(Smax):
        off.append(F)
        F += n_p[p] * D
    pieces = []  # (plane, bank, width, sbuf_col)
    for p in range(Smax):
        W = n_p[p] * D
        c0 = 0
        while c0 < W:
            w = min(BANK, W - c0)
            pieces.append((p, c0 // BANK, w, off[p] + c0))
            c0 += w
    last_touch = {}
    for i, (p, b, w, sc) in enumerate(pieces):
        last_touch[b] = i
    batches = []  # lists of piece indices, contiguous in sbuf cols
    cur, cur_w = [], 0
    for i, (p, b, w, sc) in enumerate(pieces):
        if cur and cur_w + w > BMAX:
            batches.append(cur)
            cur, cur_w = [], 0
        cur.append(i)
        cur_w += w
    if cur:
        batches.append(cur)
    return n_p, off, F, pieces, last_touch, batches


def _build(S_g, reps=1, loop_n=0):
    import concourse.bacc as bacc
    import concourse.tile as tile
    import concourse.mybir as mybir

    n_p, off, F, pieces, last_touch, batches = _plan(S_g)
    nbank = -(-YC // BANK)
    bankw = [min(BANK, YC - BANK * b) for b in range(nbank)]

    nc = bacc.Bacc("TRN2", target_bir_lowering=False, debug=False,
                   num_devices=NC)
    u8 = mybir.dt.uint8
    f8 = mybir.dt.float8e3
    f16 = mybir.dt.float16
    f32 = mybir.dt.float32

    xj = nc.dram_tensor("xj", (128, F), u8, kind="ExternalInput").ap()
    eye_d = nc.dram_tensor("eye", (128, 128), u8, kind="ExternalInput").ap()
    y = nc.dram_tensor("y", (128, YC), f16, kind="ExternalOutput").ap()

    with tile.TileContext(nc) as tc:
        with (
            tc.tile_pool(name="xt", bufs=3) as xpool,
            tc.tile_pool(name="ey", bufs=1) as epool,
            tc.psum_pool(name="ps", bufs=1) as ppool,
            tc.tile_pool(name="st", bufs=2) as spool,
        ):
            def body():
                eye = epool.tile([128, 128], u8, tag="ey")
                nc.sync.dma_start(eye[:], eye_d[:, :])
                lhsT = eye[:].bitcast(f8)
                for _ in range(reps):
                    pt = [ppool.tile([128, bankw[b]], f32, tag=f"ps{b}",
                                     name=f"ps{b}")
                          for b in range(nbank)]
                    st = spool.tile([128, YC], f16, tag="st")
                    drained = 0
                    for blist in batches:
                        c0 = pieces[blist[0]][3]
                        bw = sum(pieces[i][2] for i in blist)
                        xt = xpool.tile([128, bw], u8, tag="xt")
                        nc.sync.dma_start(xt[:], xj[:, c0:c0 + bw])
                        for i in blist:
                            (p, b, w, sc) = pieces[i]
                            rhs = xt[:, sc - c0:sc - c0 + w].bitcast(f8)
                            nc.tensor.matmul(
                                pt[b][:, 0:w], lhsT, rhs,
                                start=(p == 0), stop=(i == last_touch[b]))
                            if i == last_touch[b]:
                                dst = st[:, BANK * b:BANK * b + bankw[b]]
                                if drained % 2 == 0:
                                    nc.vector.tensor_copy(dst, pt[b][:])
                                else:
                                    nc.scalar.copy(dst, pt[b][:])
                                drained += 1
                    nc.scalar.dma_start(y[:, :], st[:])

            if loop_n:
                with tc.For_i(0, loop_n, 1,
                              hint_engines=(mybir.EngineType.DVE,)):
                    body()
            else:
                body()

    nc.compile()
    return nc


def _structure(deg_sorted):
    """deg_sorted: [NC, NNP] per-core degrees in descending order.
    Returns the cross-core padded slots per 128-node group."""
    S_g = deg_sorted[:, ::CH].max(axis=0)
    return tuple(int(s) for s in np.maximum(S_g, 1))


def _prep_inputs(x, edge_index):
    """Returns (in_maps, S_g, perms)."""
    x = np.ascontiguousarray(np.asarray(x), dtype=np.float32)
    ei = np.asarray(edge_index)
    src = ei[0].astype(np.int64)
    dst = ei[1].astype(np.int64)

    alpha = FP8_MAX / max(float(np.abs(x).max()), 1e-30)
    x8 = (x * alpha).astype(F8)

    core = dst // NPC
    per_core = []
    perms = []
    deg_sorted = np.zeros((NC, NNP), np.int64)
    for k in range(NC):
        m = core == k
        s_k = src[m]
        d_k = dst[m] - k * NPC
        deg = np.zeros(NNP, np.int64)
        deg[:NPC] = np.bincount(d_k, minlength=NPC)
        perm = np.argsort(-deg, kind="stable")   # node ids, degree desc
        deg_sorted[k] = deg[perm]
        perms.append(perm)
        per_core.append((s_k, d_k))

    S_g = _structure(deg_sorted)
    n_p, off, F, _, _, _ = _plan(S_g)
    off = np.asarray(off, np.int64)

    feat_idx = np.arange(D, dtype=np.int64)[None, :]
    in_maps = []
    eye_u8 = np.ascontiguousarray(
        np.eye(128, dtype=np.float32).astype(F8).view(np.uint8))
    for k in range(NC):
        s_k, d_k = per_core[k]
        perm = perms[k]
        pos = np.empty(NNP, np.int64)
        pos[perm] = np.arange(NNP)
        q = pos[d_k]                       # sorted position per edge
        order = np.argsort(q, kind="stable")
        qo = q[order]
        so = s_k[order]
        cnts = np.bincount(qo, minlength=NNP)
        cum = np.concatenate(([0], np.cumsum(cnts)))
        slot = np.arange(len(qo), dtype=np.int64) - cum[qo]
        u = qo % CH
        g = qo // CH
        col0 = off[slot] + g * D
        xjk = np.zeros((128, F), F8)
        xjk[u[:, None], col0[:, None] + feat_idx] = x8[so]
        in_maps.append({"xj": xjk.view(np.uint8), "eye": eye_u8})
    return in_maps, S_g, perms, alpha


def kernel(x, edge_index):
    from concourse import bass_utils

    in_maps, S_g, perms, alpha = _prep_inputs(x, edge_index)
    if S_g not in _cache:
        _cache[S_g] = _build(S_g)
    nc = _cache[S_g]

    res = None
    for attempt in range(3):
        try:
            res = bass_utils.run_bass_kernel_spmd(nc, in_maps,
                                                  core_ids=list(range(NC)))
            break
        except Exception:
            if attempt == 2:
                raise
    out = np.empty((N, D), np.float32)
    inv_alpha = np.float32(1.0 / alpha)
    for k in range(NC):
        yk = np.asarray(res.results[k]["y"]).reshape(128, NCHUNK, D)
        yk = yk.transpose(1, 0, 2).reshape(NNP, D).astype(np.float32)
        yk *= inv_alpha
        perm = perms[k]
        valid = perm < NPC
        out[k * NPC + perm[valid]] = yk[valid]
    return out
